# revision 1
# baseline (speedup 1.0000x reference)
"""Trainium2 Bass kernel for nn_BaselineModel_47682726921062.

Model: token embedding lookup -> input projection -> 512-step tanh RNN
-> softmax over the hidden dim. Output [64, 512, 512] = softmax(h, axis=1)
with h[b, :, t] the hidden state after step t.

Strategy: data-parallel over batch across 8 NeuronCores (8 examples/core),
weights replicated, zero collectives. Per core everything is kept
hidden-major ([128 partitions = hidden%128, free]):
  - embedding rows are gathered *and transposed* in one SWDGE dma_gather
    (bf16, elem 1KB), landing as xeT[p, c, (t*8+b)] = emb[x, c*128+p]
  - projection xp = W_ih @ xe^T accumulates in PSUM over 4 k-chunks,
    bias applied by ScalarE while evacuating PSUM -> SBUF (fp32)
  - recurrence: per step 16 bf16 matmuls (W_hh.T tiles stationary,
    h^T [128, 8] moving), xp added by VectorE into PSUM, Tanh by ScalarE
    back to SBUF bf16; split in two independent PSUM banks per step so
    tanh of the first half pipelines under the matmuls of the second
  - softmax over hidden: exp on ScalarE, partition+chunk sums via a
    ones-matmul accumulated over chunks, reciprocal_approx_fast,
    normalize on VectorE, DMA out per 64-timestep block.
"""

import sys

if "/opt/trn_rl_repo" not in sys.path:
    sys.path.insert(0, "/opt/trn_rl_repo")

import numpy as np
import ml_dtypes

BATCH, SEQ, VOCAB, DIM = 64, 512, 32000, 512
NCORES = 8
BC = BATCH // NCORES          # 8 examples per core
P = 128
KC = DIM // P                 # 4 chunks of 128
NIDX = SEQ * BC               # 4096 gathered rows per core
NBLK = 8                      # gather/projection blocks of 512 (t,b) columns
BLK = NIDX // NBLK            # 512
TB = 8                        # softmax/output t-blocks
TBS = SEQ // TB               # 64 timesteps per block

TRACE = False                 # test.py flips this for profiling
LAST_RESULT = None            # BassKernelResults of the last run

_cache = {}


def _build():
    import concourse.mybir as mybir
    import concourse.tile as tile
    from concourse import bacc

    f32 = mybir.dt.float32
    bf16 = mybir.dt.bfloat16

    nc = bacc.Bacc("TRN2")

    emb = nc.dram_tensor("emb", [VOCAB, DIM], bf16, kind="ExternalInput")
    idx = nc.dram_tensor("idx", [P, NIDX // 16], mybir.dt.int16, kind="ExternalInput")
    wih = nc.dram_tensor("wih", [DIM, DIM], bf16, kind="ExternalInput")   # W_ih.T
    whh = nc.dram_tensor("whh", [DIM, DIM], bf16, kind="ExternalInput")   # W_hh.T
    bias = nc.dram_tensor("bias", [P, KC], f32, kind="ExternalInput")     # (b_ih+b_hh)[c*128+p] at [p,c]
    ones = nc.dram_tensor("ones", [P, P], bf16, kind="ExternalInput")
    out = nc.dram_tensor("out", [BC, DIM, SEQ], f32, kind="ExternalOutput")

    with tile.TileContext(nc) as tc:
        with (
            tc.tile_pool(name="consts", bufs=1) as consts,
            tc.tile_pool(name="xe", bufs=2) as xe_pool,
            tc.tile_pool(name="xp", bufs=1) as xp_pool,
            tc.tile_pool(name="h", bufs=1) as h_pool,
            tc.tile_pool(name="sm", bufs=1) as sm_pool,
            tc.tile_pool(name="expb", bufs=2) as exp_pool,
            tc.tile_pool(name="stage", bufs=2) as stage_pool,
        ):
            idx_sb = consts.tile([P, NIDX // 16], mybir.dt.int16)
            nc.sync.dma_start(idx_sb[:], idx[:])
            wih_sb = consts.tile([P, KC, DIM], bf16)
            nc.sync.dma_start(wih_sb[:], wih.rearrange("(kc p) m -> p kc m", p=P))
            whh_sb = consts.tile([P, KC, DIM], bf16)
            nc.sync.dma_start(whh_sb[:], whh.rearrange("(kc p) m -> p kc m", p=P))
            bias_sb = consts.tile([P, KC], f32)
            nc.sync.dma_start(bias_sb[:], bias[:])
            ones_sb = consts.tile([P, P], bf16)
            nc.sync.dma_start(ones_sb[:], ones[:])

            xp_all = xp_pool.tile([P, KC, NIDX], f32)          # 64 KB/partition
            hT_all = h_pool.tile([P, SEQ, KC, BC], bf16)       # 32 KB/partition

            # ---- Phase A: gather + input projection ----------------------
            with nc.named_scope("proj"), tc.tile_pool(
                name="pps", bufs=4, space="PSUM"
            ) as pps:
                for nb in range(NBLK):
                    xe = xe_pool.tile([P, KC, BLK], bf16, tag="xe")
                    nc.gpsimd.dma_gather(
                        xe[:], emb[:], idx_sb[:, nb * 32 : (nb + 1) * 32],
                        num_idxs=BLK, num_idxs_reg=BLK, elem_size=DIM,
                        transpose=True,
                    )
                    for mc in range(KC):
                        ps = pps.tile([P, BLK], f32, tag="pp")
                        for kc in range(KC):
                            nc.tensor.matmul(
                                ps[:],
                                wih_sb[:, kc, mc * P : (mc + 1) * P],
                                xe[:, kc, :],
                                start=(kc == 0), stop=(kc == KC - 1),
                            )
                        nc.scalar.activation(
                            xp_all[:, mc, nb * BLK : (nb + 1) * BLK], ps[:],
                            mybir.ActivationFunctionType.Identity,
                            bias=bias_sb[:, mc : mc + 1], scale=1.0,
                        )

            # ---- Phase B: recurrence ------------------------------------
            with nc.named_scope("recurrence"), tc.tile_pool(
                name="rps", bufs=2, space="PSUM"
            ) as rps:
                # t = 0: h_0 = tanh(xp_0)
                nc.scalar.activation(
                    hT_all[:, 0, :, :], xp_all[:, :, 0:BC],
                    mybir.ActivationFunctionType.Tanh,
                )
                for t in range(1, SEQ):
                    for half in range(2):
                        ps = rps.tile([P, 2, BC], f32, tag=f"rec{half}")
                        for i2 in range(2):
                            ic = 2 * half + i2
                            for kc in range(KC):
                                nc.tensor.matmul(
                                    ps[:, i2, :],
                                    whh_sb[:, kc, ic * P : (ic + 1) * P],
                                    hT_all[:, t - 1, kc, :],
                                    start=(kc == 0), stop=(kc == KC - 1),
                                )
                        nc.vector.tensor_tensor(
                            ps[:], ps[:],
                            xp_all[:, 2 * half : 2 * half + 2, t * BC : (t + 1) * BC],
                            mybir.AluOpType.add,
                        )
                        nc.scalar.activation(
                            hT_all[:, t, 2 * half : 2 * half + 2, :], ps[:],
                            mybir.ActivationFunctionType.Tanh,
                        )

            # ---- Phase C: softmax over hidden + output ------------------
            with nc.named_scope("softmax"), tc.tile_pool(
                name="sps", bufs=2, space="PSUM"
            ) as sps:
                recip_sb = sm_pool.tile([P, SEQ, BC], f32)     # 16 KB/partition
                for tb in range(TB):
                    tsl = slice(tb * TBS, (tb + 1) * TBS)
                    ex = exp_pool.tile([P, TBS, KC, BC], bf16, tag="ex")
                    nc.scalar.activation(
                        ex[:], hT_all[:, tsl, :, :],
                        mybir.ActivationFunctionType.Exp,
                    )
                    sp = sps.tile([P, TBS, BC], f32, tag="sum")
                    for c in range(KC):
                        nc.tensor.matmul(
                            sp[:], ones_sb[:], ex[:, :, c, :],
                            start=(c == 0), stop=(c == KC - 1),
                        )
                    nc.vector.reciprocal_approx_fast(recip_sb[:, tsl, :], sp[:])
                    st = stage_pool.tile([P, KC, BC, TBS], f32, tag="st")
                    for c in range(KC):
                        nc.vector.tensor_tensor(
                            st[:, c].rearrange("p b t -> p t b"),
                            ex[:, :, c, :],
                            recip_sb[:, tsl, :],
                            mybir.AluOpType.mult,
                        )
                    for c in range(KC):
                        nc.sync.dma_start(
                            out[:, c * P : (c + 1) * P, tsl].rearrange(
                                "b p t -> p b t"
                            ),
                            st[:, c],
                        )

    nc.compile()
    return nc


def _prep_core_inputs(x_core, shared):
    flat = np.ascontiguousarray(x_core.T).reshape(-1).astype(np.int16)  # j = t*8+b
    idx = np.zeros((P, NIDX // 16), np.int16)
    for nb in range(NBLK):
        blk = flat[nb * BLK : (nb + 1) * BLK].reshape(BLK // 16, 16).T  # [16, 32]
        idx[:, nb * 32 : (nb + 1) * 32] = np.tile(blk, (P // 16, 1))
    m = dict(shared)
    m["idx"] = idx
    return m


def kernel(x, emb, W_ih, W_hh, b_ih, b_hh):
    global LAST_RESULT
    from concourse.bass_utils import run_bass_kernel_spmd

    x = np.asarray(x)
    emb = np.asarray(emb, dtype=np.float32)
    W_ih = np.asarray(W_ih, dtype=np.float32)
    W_hh = np.asarray(W_hh, dtype=np.float32)
    b_ih = np.asarray(b_ih, dtype=np.float32)
    b_hh = np.asarray(b_hh, dtype=np.float32)

    if "nc" not in _cache:
        _cache["nc"] = _build()
    nc = _cache["nc"]

    shared = {
        "emb": np.ascontiguousarray(emb).astype(ml_dtypes.bfloat16),
        "wih": np.ascontiguousarray(W_ih.T).astype(ml_dtypes.bfloat16),
        "whh": np.ascontiguousarray(W_hh.T).astype(ml_dtypes.bfloat16),
        "bias": np.ascontiguousarray((b_ih + b_hh).reshape(KC, P).T).astype(np.float32),
        "ones": np.ones((P, P), ml_dtypes.bfloat16),
    }
    in_maps = [
        _prep_core_inputs(x[c * BC : (c + 1) * BC], shared) for c in range(NCORES)
    ]
    res = run_bass_kernel_spmd(
        nc, in_maps, core_ids=list(range(NCORES)), trace=TRACE,
        **({"stitch_traces": True} if TRACE else {}),
    )
    LAST_RESULT = res
    return np.concatenate([res.results[c]["out"] for c in range(NCORES)], axis=0)


# revision 6
# speedup vs baseline: 32.5261x; 32.5261x over previous
"""Trainium2 Bass kernel for nn_BaselineModel_47682726921062.

Model: token embedding lookup -> input projection -> 512-step tanh RNN
-> softmax over the hidden dim. Output [64, 512, 512] = softmax(h, axis=1)
with h[b, :, t] the hidden state after step t.

Strategy: data-parallel over batch across 8 NeuronCores (8 examples/core),
weights replicated, zero collectives. Per core everything is kept
hidden-major ([128 partitions = hidden%128, free]):
  - embedding rows are gathered *and transposed* in one SWDGE dma_gather
    (bf16, elem 1KB), landing as xeT[p, c, (t*8+b)] = emb[x, c*128+p]
  - projection xp = W_ih @ xe^T accumulates in PSUM over 4 k-chunks,
    bias applied by ScalarE while evacuating PSUM -> SBUF (fp32)
  - recurrence: per step 16 bf16 matmuls (W_hh.T tiles stationary,
    h^T [128, 8] moving), xp added by VectorE into PSUM, Tanh by ScalarE
    back to SBUF bf16; split in two independent PSUM banks per step so
    tanh of the first half pipelines under the matmuls of the second
  - softmax over hidden: exp on ScalarE, partition+chunk sums via a
    ones-matmul accumulated over chunks, reciprocal_approx_fast,
    normalize on VectorE, DMA out per 64-timestep block.
"""

import sys

if "/opt/trn_rl_repo" not in sys.path:
    sys.path.insert(0, "/opt/trn_rl_repo")

import numpy as np
import ml_dtypes

BATCH, SEQ, VOCAB, DIM = 64, 512, 32000, 512
NCORES = 8
BC = BATCH // NCORES          # 8 examples per core
P = 128
KC = DIM // P                 # 4 chunks of 128
NIDX = SEQ * BC               # 4096 gathered rows per core
NBLK = 8                      # gather/projection blocks of 512 (t,b) columns
BLK = NIDX // NBLK            # 512
TB = 8                        # softmax/output t-blocks
TBS = SEQ // TB               # 64 timesteps per block

TRACE = False                 # test.py flips this for profiling
LAST_RESULT = None            # BassKernelResults of the last run
REC_ORDER = "kc_outer_half"   # "ic_outer" | "kc_outer_half" | "single"
REC_PSUM_BUFS = 3

_cache = {}


def _build():
    import concourse.mybir as mybir
    import concourse.tile as tile
    from concourse import bacc

    f32 = mybir.dt.float32
    bf16 = mybir.dt.bfloat16

    nc = bacc.Bacc("TRN2")

    emb = nc.dram_tensor("emb", [VOCAB, DIM], bf16, kind="ExternalInput")
    idx = nc.dram_tensor("idx", [P, NIDX // 16], mybir.dt.int16, kind="ExternalInput")
    wih = nc.dram_tensor("wih", [DIM, DIM], bf16, kind="ExternalInput")   # W_ih.T
    whh = nc.dram_tensor("whh", [DIM, DIM], bf16, kind="ExternalInput")   # W_hh.T
    bias = nc.dram_tensor("bias", [P, KC], f32, kind="ExternalInput")     # (b_ih+b_hh)[c*128+p] at [p,c]
    ones = nc.dram_tensor("ones", [P, P], bf16, kind="ExternalInput")
    out = nc.dram_tensor("out", [BC, DIM, SEQ], f32, kind="ExternalOutput")

    with tile.TileContext(nc) as tc:
        with (
            tc.tile_pool(name="consts", bufs=1) as consts,
            tc.tile_pool(name="xe", bufs=2) as xe_pool,
            tc.tile_pool(name="xp", bufs=1) as xp_pool,
            tc.tile_pool(name="h", bufs=1) as h_pool,
            tc.tile_pool(name="sm", bufs=1) as sm_pool,
            tc.tile_pool(name="expb", bufs=2) as exp_pool,
            tc.tile_pool(name="stage", bufs=2) as stage_pool,
        ):
            idx_sb = consts.tile([P, NIDX // 16], mybir.dt.int16)
            nc.sync.dma_start(idx_sb[:], idx[:])
            wih_sb = consts.tile([P, KC, DIM], bf16)
            nc.sync.dma_start(wih_sb[:], wih.rearrange("(kc p) m -> p kc m", p=P))
            whh_sb = consts.tile([P, KC, DIM], bf16)
            nc.sync.dma_start(whh_sb[:], whh.rearrange("(kc p) m -> p kc m", p=P))
            bias_sb = consts.tile([P, KC], f32)
            nc.sync.dma_start(bias_sb[:], bias[:])
            ones_sb = consts.tile([P, P], bf16)
            nc.sync.dma_start(ones_sb[:], ones[:])

            xp_all = xp_pool.tile([P, KC, NIDX], f32)          # 64 KB/partition
            hT_all = h_pool.tile([P, SEQ, KC, BC], bf16)       # 32 KB/partition

            # ---- Phase A: gather + input projection ----------------------
            with nc.named_scope("proj"), tc.tile_pool(
                name="pps", bufs=4, space="PSUM"
            ) as pps:
                for nb in range(NBLK):
                    xe = xe_pool.tile([P, KC, BLK], bf16, tag="xe")
                    nc.gpsimd.dma_gather(
                        xe[:], emb[:], idx_sb[:, nb * 32 : (nb + 1) * 32],
                        num_idxs=BLK, num_idxs_reg=BLK, elem_size=DIM,
                        transpose=True,
                    )
                    for mc in range(KC):
                        ps = pps.tile([P, BLK], f32, tag="pp")
                        for kc in range(KC):
                            nc.tensor.matmul(
                                ps[:],
                                wih_sb[:, kc, mc * P : (mc + 1) * P],
                                xe[:, kc, :],
                                start=(kc == 0), stop=(kc == KC - 1),
                            )
                        nc.scalar.activation(
                            xp_all[:, mc, nb * BLK : (nb + 1) * BLK], ps[:],
                            mybir.ActivationFunctionType.Identity,
                            bias=bias_sb[:, mc : mc + 1], scale=1.0,
                        )

            # ---- Phase B: recurrence ------------------------------------
            with nc.named_scope("recurrence"), tc.tile_pool(
                name="rps", bufs=REC_PSUM_BUFS, space="PSUM"
            ) as rps:
                # t = 0: h_0 = tanh(xp_0)
                nc.scalar.activation(
                    hT_all[:, 0, :, :], xp_all[:, :, 0:BC],
                    mybir.ActivationFunctionType.Tanh,
                )
                for t in range(1, SEQ):
                    if REC_ORDER == "single":
                        ps = rps.tile([P, KC, BC], f32, tag="rec")
                        first = True
                        for kc in range(KC):
                            for ic in range(KC):
                                nc.tensor.matmul(
                                    ps[:, ic, :],
                                    whh_sb[:, kc, ic * P : (ic + 1) * P],
                                    hT_all[:, t - 1, kc, :],
                                    start=first,
                                    stop=(kc == KC - 1 and ic == KC - 1),
                                    skip_group_check=True,
                                )
                                first = False
                        nc.vector.tensor_tensor(
                            ps[:], ps[:],
                            xp_all[:, :, t * BC : (t + 1) * BC],
                            mybir.AluOpType.add,
                        )
                        nc.scalar.activation(
                            hT_all[:, t, :, :], ps[:],
                            mybir.ActivationFunctionType.Tanh,
                        )
                        continue
                    for half in range(2):
                        ps = rps.tile([P, 2, BC], f32, tag=f"rec{half}")
                        if REC_ORDER == "kc_outer_half":
                            # kc-outer within the half: the first MMs consume
                            # only h-chunks 0,1 (the other half's tanh gets
                            # time to land), accumulation groups interleave
                            # via per-element has_written semantics.
                            first = True
                            for kc in range(KC):
                                for i2 in range(2):
                                    ic = 2 * half + i2
                                    nc.tensor.matmul(
                                        ps[:, i2, :],
                                        whh_sb[:, kc, ic * P : (ic + 1) * P],
                                        hT_all[:, t - 1, kc, :],
                                        start=first, stop=(kc == KC - 1 and i2 == 1),
                                        skip_group_check=True,
                                    )
                                    first = False
                        else:
                            for i2 in range(2):
                                ic = 2 * half + i2
                                for kc in range(KC):
                                    nc.tensor.matmul(
                                        ps[:, i2, :],
                                        whh_sb[:, kc, ic * P : (ic + 1) * P],
                                        hT_all[:, t - 1, kc, :],
                                        start=(kc == 0), stop=(kc == KC - 1),
                                    )
                        nc.vector.tensor_tensor(
                            ps[:], ps[:],
                            xp_all[:, 2 * half : 2 * half + 2, t * BC : (t + 1) * BC],
                            mybir.AluOpType.add,
                        )
                        nc.scalar.activation(
                            hT_all[:, t, 2 * half : 2 * half + 2, :], ps[:],
                            mybir.ActivationFunctionType.Tanh,
                        )

            # ---- Phase C: softmax over hidden + output ------------------
            with nc.named_scope("softmax"), tc.tile_pool(
                name="sps", bufs=2, space="PSUM"
            ) as sps:
                recip_sb = sm_pool.tile([P, SEQ, BC], f32)     # 16 KB/partition
                for tb in range(TB):
                    tsl = slice(tb * TBS, (tb + 1) * TBS)
                    ex = exp_pool.tile([P, TBS, KC, BC], bf16, tag="ex")
                    nc.scalar.activation(
                        ex[:], hT_all[:, tsl, :, :],
                        mybir.ActivationFunctionType.Exp,
                    )
                    sp = sps.tile([P, TBS, BC], f32, tag="sum")
                    for c in range(KC):
                        nc.tensor.matmul(
                            sp[:], ones_sb[:], ex[:, :, c, :],
                            start=(c == 0), stop=(c == KC - 1),
                        )
                    nc.vector.reciprocal_approx_fast(recip_sb[:, tsl, :], sp[:])
                    st = stage_pool.tile([P, KC, BC, TBS], f32, tag="st")
                    for c in range(KC):
                        nc.vector.tensor_tensor(
                            st[:, c].rearrange("p b t -> p t b"),
                            ex[:, :, c, :],
                            recip_sb[:, tsl, :],
                            mybir.AluOpType.mult,
                        )
                    for c in range(KC):
                        nc.sync.dma_start(
                            out[:, c * P : (c + 1) * P, tsl].rearrange(
                                "b p t -> p b t"
                            ),
                            st[:, c],
                        )

    nc.compile()
    return nc


def _prep_core_inputs(x_core, shared):
    flat = np.ascontiguousarray(x_core.T).reshape(-1).astype(np.int16)  # j = t*8+b
    idx = np.zeros((P, NIDX // 16), np.int16)
    for nb in range(NBLK):
        blk = flat[nb * BLK : (nb + 1) * BLK].reshape(BLK // 16, 16).T  # [16, 32]
        idx[:, nb * 32 : (nb + 1) * 32] = np.tile(blk, (P // 16, 1))
    m = dict(shared)
    m["idx"] = idx
    return m


def kernel(x, emb, W_ih, W_hh, b_ih, b_hh):
    global LAST_RESULT
    from concourse.bass_utils import run_bass_kernel_spmd

    x = np.asarray(x)
    emb = np.asarray(emb, dtype=np.float32)
    W_ih = np.asarray(W_ih, dtype=np.float32)
    W_hh = np.asarray(W_hh, dtype=np.float32)
    b_ih = np.asarray(b_ih, dtype=np.float32)
    b_hh = np.asarray(b_hh, dtype=np.float32)

    if "nc" not in _cache:
        _cache["nc"] = _build()
    nc = _cache["nc"]

    shared = {
        "emb": np.ascontiguousarray(emb).astype(ml_dtypes.bfloat16),
        "wih": np.ascontiguousarray(W_ih.T).astype(ml_dtypes.bfloat16),
        "whh": np.ascontiguousarray(W_hh.T).astype(ml_dtypes.bfloat16),
        "bias": np.ascontiguousarray((b_ih + b_hh).reshape(KC, P).T).astype(np.float32),
        "ones": np.ones((P, P), ml_dtypes.bfloat16),
    }
    in_maps = [
        _prep_core_inputs(x[c * BC : (c + 1) * BC], shared) for c in range(NCORES)
    ]
    res = run_bass_kernel_spmd(
        nc, in_maps, core_ids=list(range(NCORES)), trace=TRACE,
        **({"stitch_traces": True} if TRACE else {}),
    )
    LAST_RESULT = res
    return np.concatenate([res.results[c]["out"] for c in range(NCORES)], axis=0)
